# revision 28
# baseline (speedup 1.0000x reference)
"""MLA attention kernel for 8 Trainium2 NeuronCores.

Sharding: core i -> batch b = i//4, head group hg = i%4 (32 heads each).
Latent down-projections replicated within a batch group; Wq_up/Wq_rope/
Wk_up/Wv_up/Wo sharded by head.  Host sums the 4 partial outputs per batch.

Device program (identical on all cores, SPMD over different data):
  - all matmuls bf16 with fp32 PSUM accumulation
  - projections computed feature-major (features on partitions) so that
    attention scores S^T[k, q] = kT.T @ qT need no transposes
  - softmax: exp on ScalarE (scale 1/sqrt(96) folded in, no max subtraction:
    scores are ~N(0,1)), denominator via an appended ones-column of V in the
    attn@V matmul, division via DVE fast-reciprocal + gpsimd broadcast
  - the ScalarE exp stream (≈11.4us/head) exceeds the PE's own attention
    work (≈7us/head), so group g+1's projection matmuls are interleaved
    2-at-a-time between the score matmuls of group g to keep the PE fed
    while ACT drains; Wo slabs prefetch into the same slab ring during the
    last group and phase E writes PSUM->HBM directly.
"""

import sys

sys.path.insert(0, "/opt/trn_rl_repo")

import numpy as np
import ml_dtypes

import concourse.bass as bass
import concourse.tile as tile
from concourse import bacc, mybir
from concourse.bass_utils import run_bass_kernel_spmd

P = 128
T = 1024          # tokens per batch
DM = 4096         # d_model
KX = DM // P      # 32 feature chunks of x
LAT = 512         # latent dim
LC = LAT // P     # 4 latent chunks
NHC = 32          # heads per core
DH = 32           # head dim (compressed part)
DR = 64           # rope dim per head
NB = 2            # batch
SCALE = 1.0 / float(np.sqrt(DH + DR))

BF = mybir.dt.bfloat16
F32 = mybir.dt.float32

_CACHE = {}


def _build_program():
    nc = bacc.Bacc("TRN2", target_bir_lowering=False, num_devices=8)

    xT = nc.declare_dram_parameter("xT", [DM, T], BF, isOutput=False)
    wqd = nc.declare_dram_parameter("wqd", [DM, LAT], BF, isOutput=False)
    wkvd = nc.declare_dram_parameter("wkvd", [DM, LAT], BF, isOutput=False)
    wqu = nc.declare_dram_parameter("wqu", [LAT, NHC * DH], BF, isOutput=False)
    wku = nc.declare_dram_parameter("wku", [LAT, NHC * DH], BF, isOutput=False)
    wvu = nc.declare_dram_parameter("wvu", [LAT, NHC * DH], BF, isOutput=False)
    wqr = nc.declare_dram_parameter("wqr", [DM, NHC * DR], BF, isOutput=False)
    wkr = nc.declare_dram_parameter("wkr", [DM, DR], BF, isOutput=False)
    wo = nc.declare_dram_parameter("wo", [NHC * DH, DM], BF, isOutput=False)
    # bf16 output halves the writeback bytes; host accumulates the four
    # per-core partials in fp32 (adds ~0.2% rounding vs a 2e-2 budget)
    out = nc.declare_dram_parameter("out", [T, DM], BF, isOutput=True)

    from contextlib import ExitStack

    with tile.TileContext(nc) as tc, ExitStack() as octx:
        const = octx.enter_context(tc.tile_pool(name="const", bufs=1))

        xT_sb = const.tile([P, KX, T], BF, name="xT_sb")
        xT_r = xT[:].rearrange("(ko p) t -> p ko t", p=P)
        wkr_sb = const.tile([P, KX, DR], BF, name="wkr_sb")
        wvu_sb = const.tile([P, LC, NHC * DH], BF, name="wvu_sb")

        cq_sb = const.tile([P, LC, T], BF, name="cq_sb")      # c_q^T
        ckv_sb = const.tile([P, LC, T], BF, name="ckv_sb")    # c_kv^T
        kr_sb = const.tile([DR, T], BF, name="kr_sb")         # k_rope^T (shared)
        # v token-major, per (key-chunk, head): cols 0:32 = v, col 32 = ones
        v_sb = const.tile([P, 8, NHC, 34], BF, name="v_sb")
        # attention output, feature-major: head h -> [32*(h%4):.., h//4, :]
        aout_sb = const.tile([P, 8, T], BF, name="aout_sb")

        nc.vector.memset(v_sb[:, :, :, 32:33], 1.0)

        with ExitStack() as ctx:
            wpool = ctx.enter_context(tc.tile_pool(name="wpool", bufs=2))
            qkpool = ctx.enter_context(tc.tile_pool(name="qkpool", bufs=14))
            ppool = ctx.enter_context(tc.tile_pool(name="ppool", bufs=3))
            rpool = ctx.enter_context(tc.tile_pool(name="rpool", bufs=2))

            def big_slab(wd, m, name, eng=None):
                """[P, KX, P] slab of a [DM, *] weight, DMA split in two.

                eng picks the HWDGE queue: phase-B slabs ride the scalar
                engine's queue so they don't serialize behind the 8MB xT
                stream on the sync queue."""
                e = eng if eng is not None else nc.sync
                t = wpool.tile([P, KX, P], BF, tag="wqrs", name=name)
                src = wd[:, m * P:(m + 1) * P].rearrange("(ko p) m -> p ko m", p=P)
                e.dma_start(out=t[:, 0:16, :], in_=src[:, 0:16, :])
                e.dma_start(out=t[:, 16:KX, :], in_=src[:, 16:KX, :])
                return t

            def wo_slab(n):
                t = wpool.tile([P, 8, 512], BF, tag="wqrs", name=f"wos{n}")
                src = wo[:, n * 512:(n + 1) * 512].rearrange(
                    "(kc p) m -> p kc m", p=P
                )
                nc.sync.dma_start(out=t[:, 0:4, :], in_=src[:, 0:4, :])
                nc.sync.dma_start(out=t[:, 4:8, :], in_=src[:, 4:8, :])
                return t

            with ExitStack() as pctx:
                cpp = pctx.enter_context(
                    tc.tile_pool(name="cpp", bufs=2, space="PSUM")
                )
                spp = pctx.enter_context(
                    tc.tile_pool(name="spp", bufs=4, space="PSUM")
                )
                avp = pctx.enter_context(
                    tc.tile_pool(name="avp", bufs=2, space="PSUM")
                )

                # ---- Phase B: latent down-projections (feature-major) ----
                # Slab DMAs are interleaved between the xT chunk DMAs on the
                # sync queue: slab i is only needed ~i*14us in, so this keeps
                # the xT stream (which gates every matmul) almost unimpeded
                # while each slab still arrives before its chain starts.
                # Chains run in PAIRS so each xT chunk feeds 4 matmuls
                # (~0.87us demand vs ~0.8us supply) - no PE starvation while
                # xT is still streaming.
                items = [(wqd, cq_sb, m) for m in range(LC)] + [
                    (wkvd, ckv_sb, m) for m in range(LC)
                ]
                slabs = {0: big_slab(items[0][0], items[0][2], "bs0")}
                for kq in range(4):
                    nc.sync.dma_start(
                        out=xT_sb[:, kq * 8:(kq + 1) * 8, :],
                        in_=xT_r[:, kq * 8:(kq + 1) * 8, :],
                    )
                    nwd, _, nm = items[kq + 1]
                    slabs[kq + 1] = big_slab(nwd, nm, f"bs{kq + 1}")
                nc.sync.dma_start(
                    out=wkr_sb[:], in_=wkr[:].rearrange("(ko p) d -> p ko d", p=P)
                )
                nc.sync.dma_start(
                    out=wvu_sb[:], in_=wvu[:].rearrange("(c p) m -> p c m", p=P)
                )
                for i in range(5, 8):
                    nwd, _, nm = items[i]
                    slabs[i] = big_slab(nwd, nm, f"bs{i}")
                for pi in range(4):
                    pr = (items[2 * pi], items[2 * pi + 1])
                    pss4 = [
                        (spp if ci else cpp).tile(
                            [P, 512], F32, tag=("sps" if ci else "cps"),
                            name=f"b_ps{ci}_{hf}",
                        )
                        for ci in range(2)
                        for hf in range(2)
                    ]
                    for k in range(KX):
                        for ci in range(2):
                            for hf in range(2):
                                nc.tensor.matmul(
                                    pss4[ci * 2 + hf][:],
                                    slabs[2 * pi + ci][:, k, :],
                                    xT_sb[:, k, hf * 512:(hf + 1) * 512],
                                    start=(k == 0),
                                    stop=(k == KX - 1),
                                )
                    for ci in range(2):
                        _, cdst, m = pr[ci]
                        for hf in range(2):
                            nc.vector.tensor_copy(
                                out=cdst[:, m, hf * 512:(hf + 1) * 512],
                                in_=pss4[ci * 2 + hf][:],
                            )

                # k_rope^T [64, T]: the two token halves run as concurrent
                # col-tiles (M=64 at array cols 0 and 64) in one PSUM bank
                ps = cpp.tile([P, 512], F32, tag="cps", name="kr_ps")
                for k in range(KX):
                    for hf in range(2):
                        nc.tensor.matmul(
                            ps[hf * 64:hf * 64 + DR, :],
                            wkr_sb[:, k, :],
                            xT_sb[:, k, hf * 512:(hf + 1) * 512],
                            start=(k == 0),
                            stop=(k == KX - 1),
                            tile_position=(0, hf * 64),
                        )
                for hf in range(2):
                    nc.vector.tensor_copy(
                        out=kr_sb[:, hf * 512:(hf + 1) * 512],
                        in_=ps[hf * 64:hf * 64 + DR, :],
                    )

                # ---- Phase V: v = c_kv @ Wv_up (token-major) ----
                for tt in range(8):
                    for hf in range(2):
                        ps = cpp.tile([P, 512], F32, tag="cps", name="v_ps")
                        for lc in range(LC):
                            nc.tensor.matmul(
                                ps[:],
                                ckv_sb[:, lc, tt * P:(tt + 1) * P],
                                wvu_sb[:, lc, hf * 512:(hf + 1) * 512],
                                start=(lc == 0),
                                stop=(lc == LC - 1),
                            )
                        nc.vector.tensor_copy(
                            out=v_sb[:, tt, hf * 16:(hf + 1) * 16, 0:32],
                            in_=ps[:].rearrange("p (h d) -> p h d", h=16),
                        )

                # ---- projections for one head group, as a generator that
                # yields every ~2 matmuls (72 yields) so attention of group
                # g-1 can interleave them between its score matmuls ----
                def proj_gen(g, qt, kt):
                    slab = big_slab(wqr, 2 * g, f"qrs{g}_0")
                    nxt = None
                    for s in range(2):
                        qa = qkpool.tile([P, T], BF, tag="qkt", name=f"qt{g}_{2 * s}")
                        qb = qkpool.tile(
                            [P, T], BF, tag="qkt", name=f"qt{g}_{2 * s + 1}"
                        )
                        qt.append(qa)
                        qt.append(qb)
                        for hf in range(2):
                            ps = cpp.tile([P, 512], F32, tag="cps", name="qr_ps")
                            for k0 in range(0, KX, 2):
                                for k in (k0, k0 + 1):
                                    nc.tensor.matmul(
                                        ps[:],
                                        slab[:, k, :],
                                        xT_sb[:, k, hf * 512:(hf + 1) * 512],
                                        start=(k == 0),
                                        stop=(k == KX - 1),
                                    )
                                if s == 0 and hf == 1 and k0 == 16:
                                    nxt = big_slab(wqr, 2 * g + 1, f"qrs{g}_1")
                                yield
                            sl = slice(hf * 512, (hf + 1) * 512)
                            nc.vector.tensor_copy(out=qa[0:DR, sl], in_=ps[0:DR, :])
                            nc.vector.tensor_copy(out=qb[0:DR, sl], in_=ps[DR:P, :])
                        slab = nxt

                    for idx, (wu, dst, csrc) in enumerate(
                        ((wqu, qt, cq_sb), (wku, kt, ckv_sb))
                    ):
                        if idx == 1:
                            for j in range(4):
                                ktj = qkpool.tile(
                                    [P, T], BF, tag="qkt", name=f"kt{g}_{j}"
                                )
                                kt.append(ktj)
                                # SBUF->SBUF broadcast of the shared k_rope on
                                # the DMA queue: keeps the DVE free at group
                                # seams where the first scores of the next
                                # group wait on these tiles
                                nc.sync.dma_start(out=ktj[0:DR, :], in_=kr_sb[:])
                        ups = wpool.tile([P, LC, P], BF, tag="wups", name=f"up{g}_{idx}")
                        nc.sync.dma_start(
                            out=ups[:],
                            in_=wu[:, g * P:(g + 1) * P].rearrange(
                                "(c p) m -> p c m", p=P
                            ),
                        )
                        pss = []
                        for hf in range(2):
                            ps = cpp.tile([P, 512], F32, tag="cps", name="up_ps")
                            pss.append(ps)
                            for lc in range(LC):
                                nc.tensor.matmul(
                                    ps[:],
                                    ups[:, lc, :],
                                    csrc[:, lc, hf * 512:(hf + 1) * 512],
                                    start=(lc == 0),
                                    stop=(lc == LC - 1),
                                )
                                if lc == 1:
                                    yield
                            yield
                        # copies ordered head-first so head 0's operands are
                        # complete earliest (they gate the next group's first
                        # score matmuls)
                        for j in range(4):
                            for hf in range(2):
                                nc.vector.tensor_copy(
                                    out=dst[j][DR:DR + DH,
                                               hf * 512:(hf + 1) * 512],
                                    in_=pss[hf][j * DH:(j + 1) * DH, :],
                                )

                # ---- attention, with next group's projections interleaved ----
                qts = {0: []}
                kts = {0: []}
                g0 = proj_gen(0, qts[0], kts[0])
                for _ in g0:
                    pass
                gen = None
                woslabs = []
                for g in range(8):
                    if g + 1 < 8:
                        qts[g + 1] = []
                        kts[g + 1] = []
                        gen = proj_gen(g + 1, qts[g + 1], kts[g + 1])
                    else:
                        gen = None
                        # prefetch first two Wo slabs during the last group
                        woslabs = [wo_slab(0), wo_slab(1)]

                    def pull():
                        if gen is not None:
                            next(gen, None)

                    qt, kt = qts[g], kts[g]
                    for j in range(4):
                        h = 4 * g + j
                        probs = [
                            ppool.tile(
                                [P, 8, 512], BF, tag="probs", name=f"pb{g}_{j}_{qh}"
                            )
                            for qh in range(2)
                        ]
                        for qh in range(2):
                            for kc in range(8):
                                sp = spp.tile([P, 512], F32, tag="sps", name="sps")
                                nc.tensor.matmul(
                                    sp[:],
                                    kt[j][0:96, kc * P:(kc + 1) * P],
                                    qt[j][0:96, qh * 512:(qh + 1) * 512],
                                    start=True,
                                    stop=True,
                                )
                                pull()
                                nc.scalar.activation(
                                    out=probs[qh][:, kc, :],
                                    in_=sp[:],
                                    func=mybir.ActivationFunctionType.Exp,
                                    scale=SCALE,
                                )
                        # attn@V for both query halves concurrently: the two
                        # M=33 accumulation chains col-tile the PE array
                        # (cols 0-32 and 64-96), sharing one PSUM bank.
                        av = avp.tile([P, 512], F32, tag="avp", name="av")
                        for kc in range(8):
                            for qh in range(2):
                                nc.tensor.matmul(
                                    av[qh * 64:qh * 64 + 33, :],
                                    v_sb[:, kc, h, 0:33],
                                    probs[qh][:, kc, :],
                                    start=(kc == 0),
                                    stop=(kc == 7),
                                    tile_position=(0, qh * 64),
                                )
                            if kc % 4 == 3:
                                pull()
                        for qh in range(2):
                            # ACT stages the denominator to SBUF partition 0:
                            # reciprocal_approx_fast requires base partition 0,
                            # and gpsimd (broadcast) cannot read PSUM.
                            den = rpool.tile([1, 512], F32, tag="rc", name="den")
                            nc.scalar.copy(out=den[:], in_=av[qh * 64 + 32:qh * 64 + 33, :])
                            recip = rpool.tile([1, 512], F32, tag="rc", name="recip")
                            nc.vector.reciprocal_approx_fast(recip[:], den[:])
                            rrep = rpool.tile([DH, 512], F32, tag="rr", name="rrep")
                            nc.gpsimd.partition_broadcast(rrep[:], recip[:])
                            nc.vector.tensor_mul(
                                out=aout_sb[
                                    j * DH:(j + 1) * DH, g, qh * 512:(qh + 1) * 512
                                ],
                                in0=av[qh * 64:qh * 64 + DH, :],
                                in1=rrep[:],
                            )
                            pull()
                    if gen is not None:
                        for _ in gen:
                            pass

            # ---- Phase E: out = aout^T @ Wo (token-major), PSUM->HBM direct
            with ExitStack() as ectx:
                epp = ectx.enter_context(
                    tc.tile_pool(name="epp", bufs=8, space="PSUM")
                )
                slabs = list(woslabs)
                for n in range(8):
                    t = slabs[n]
                    pss = [
                        epp.tile([P, 512], F32, tag="eps", name=f"eps_{n}_{i}")
                        for i in range(8)
                    ]
                    for tt in range(8):
                        for kc in range(8):
                            nc.tensor.matmul(
                                pss[tt][:],
                                aout_sb[:, kc, tt * P:(tt + 1) * P],
                                t[:, kc, :],
                                start=(kc == 0),
                                stop=(kc == 7),
                            )
                        # drain each PSUM bank as soon as its chain stops so
                        # the next slab's banks free up early (shorter tail)
                        ot = ppool.tile([P, 512], BF, tag="probs", name=f"eo{n}_{tt}")
                        nc.any.tensor_copy(out=ot[:], in_=pss[tt][:])
                        # scalar HWDGE queue: ACT is idle during phase E and
                        # this keeps the sync queue free for Wo slab loads
                        nc.scalar.dma_start(
                            out=out[tt * P:(tt + 1) * P, n * 512:(n + 1) * 512],
                            in_=ot[:],
                        )
                        if tt == 0 and n + 2 < 8:
                            slabs.append(wo_slab(n + 2))

    nc.compile()
    return nc


def _prep_inputs(inputs):
    bf = ml_dtypes.bfloat16
    x = np.asarray(inputs["x"], dtype=np.float32)
    Wq_down = np.asarray(inputs["Wq_down"], dtype=np.float32).astype(bf)
    Wkv_down = np.asarray(inputs["Wkv_down"], dtype=np.float32).astype(bf)
    Wq_up = np.asarray(inputs["Wq_up"], dtype=np.float32).astype(bf)
    Wk_up = np.asarray(inputs["Wk_up"], dtype=np.float32).astype(bf)
    Wv_up = np.asarray(inputs["Wv_up"], dtype=np.float32).astype(bf)
    Wq_rope = np.asarray(inputs["Wq_rope"], dtype=np.float32).astype(bf)
    Wk_rope = np.asarray(inputs["Wk_rope"], dtype=np.float32).astype(bf)
    Wo = np.asarray(inputs["Wo"], dtype=np.float32).astype(bf)

    xT = [np.ascontiguousarray(x[b].T).astype(bf) for b in range(NB)]

    in_maps = []
    for core in range(8):
        b = core // 4
        hg = core % 4
        hs = slice(hg * NHC * DH, (hg + 1) * NHC * DH)        # head-dim cols
        rs = slice(hg * NHC * DR, (hg + 1) * NHC * DR)        # rope cols
        in_maps.append(
            {
                "xT": xT[b],
                "wqd": Wq_down,
                "wkvd": Wkv_down,
                "wqu": np.ascontiguousarray(Wq_up[:, hs]),
                "wku": np.ascontiguousarray(Wk_up[:, hs]),
                "wvu": np.ascontiguousarray(Wv_up[:, hs]),
                "wqr": np.ascontiguousarray(Wq_rope[:, rs]),
                "wkr": Wk_rope,
                "wo": np.ascontiguousarray(Wo[hs, :]),
            }
        )
    return in_maps


def kernel(**inputs):
    if "nc" not in _CACHE:
        _CACHE["nc"] = _build_program()
    nc = _CACHE["nc"]
    in_maps = _prep_inputs(inputs)
    res = run_bass_kernel_spmd(nc, in_maps, list(range(8)))
    out = np.zeros((NB, T, DM), dtype=np.float32)
    for core in range(8):
        out[core // 4] += res.results[core]["out"].astype(np.float32)
    return out



# revision 33
# speedup vs baseline: 1.2112x; 1.2112x over previous
"""MLA attention kernel for 8 Trainium2 NeuronCores.

Sharding: core i -> batch b = i//4, head group hg = i%4 (32 heads each).
Latent down-projections replicated within a batch group; Wq_up/Wq_rope/
Wk_up/Wv_up/Wo sharded by head.  Host sums the 4 partial outputs per batch.

Device program (identical on all cores, SPMD over different data):
  - all matmuls bf16 with fp32 PSUM accumulation
  - projections computed feature-major (features on partitions) so that
    attention scores S^T[k, q] = kT.T @ qT need no transposes
  - softmax: exp on ScalarE (scale 1/sqrt(96) folded in, no max subtraction:
    scores are ~N(0,1)), denominator via an appended ones-column of V in the
    attn@V matmul, division via DVE fast-reciprocal + gpsimd broadcast
  - the ScalarE exp stream (≈11.4us/head) exceeds the PE's own attention
    work (≈7us/head), so group g+1's projection matmuls are interleaved
    2-at-a-time between the score matmuls of group g to keep the PE fed
    while ACT drains; Wo slabs prefetch into the same slab ring during the
    last group and phase E writes PSUM->HBM directly.
"""

import sys

sys.path.insert(0, "/opt/trn_rl_repo")

import numpy as np
import ml_dtypes

import concourse.bass as bass
import concourse.tile as tile
from concourse import bacc, mybir
from concourse.bass_utils import run_bass_kernel_spmd

P = 128
T = 1024          # tokens per batch
DM = 4096         # d_model
KX = DM // P      # 32 feature chunks of x
LAT = 512         # latent dim
LC = LAT // P     # 4 latent chunks
NHC = 32          # heads per core
DH = 32           # head dim (compressed part)
DR = 64           # rope dim per head
NB = 2            # batch
SCALE = 1.0 / float(np.sqrt(DH + DR))

BF = mybir.dt.bfloat16
F32 = mybir.dt.float32

_CACHE = {}


def _build_program():
    nc = bacc.Bacc("TRN2", target_bir_lowering=False, num_devices=8)

    xT = nc.declare_dram_parameter("xT", [DM, T], BF, isOutput=False)
    wqd = nc.declare_dram_parameter("wqd", [DM, LAT], BF, isOutput=False)
    wkvd = nc.declare_dram_parameter("wkvd", [DM, LAT], BF, isOutput=False)
    wqu = nc.declare_dram_parameter("wqu", [LAT, NHC * DH], BF, isOutput=False)
    wku = nc.declare_dram_parameter("wku", [LAT, NHC * DH], BF, isOutput=False)
    wvu = nc.declare_dram_parameter("wvu", [LAT, NHC * DH], BF, isOutput=False)
    wqr = nc.declare_dram_parameter("wqr", [DM, NHC * DR], BF, isOutput=False)
    wkr = nc.declare_dram_parameter("wkr", [DM, DR], BF, isOutput=False)
    wo = nc.declare_dram_parameter("wo", [NHC * DH, DM], BF, isOutput=False)
    # bf16 output halves the writeback bytes; host accumulates the four
    # per-core partials in fp32 (adds ~0.2% rounding vs a 2e-2 budget)
    out = nc.declare_dram_parameter("out", [T, DM], BF, isOutput=True)

    from contextlib import ExitStack

    with tile.TileContext(nc) as tc, ExitStack() as octx:
        const = octx.enter_context(tc.tile_pool(name="const", bufs=1))

        xT_sb = const.tile([P, KX, T], BF, name="xT_sb")
        xT_r = xT[:].rearrange("(ko p) t -> p ko t", p=P)
        wkr_sb = const.tile([P, KX, DR], BF, name="wkr_sb")
        wvu_sb = const.tile([P, LC, NHC * DH], BF, name="wvu_sb")

        cq_sb = const.tile([P, LC, T], BF, name="cq_sb")      # c_q^T
        ckv_sb = const.tile([P, LC, T], BF, name="ckv_sb")    # c_kv^T
        kr_sb = const.tile([DR, T], BF, name="kr_sb")         # k_rope^T (shared)
        # v token-major, per (key-chunk, head): cols 0:32 = v, col 32 = ones
        v_sb = const.tile([P, 8, NHC, 34], BF, name="v_sb")
        # attention output, feature-major: head h -> [32*(h%4):.., h//4, :]
        aout_sb = const.tile([P, 8, T], BF, name="aout_sb")

        nc.vector.memset(v_sb[:, :, :, 32:33], 1.0)

        with ExitStack() as ctx:
            wpool = ctx.enter_context(tc.tile_pool(name="wpool", bufs=2))
            qkpool = ctx.enter_context(tc.tile_pool(name="qkpool", bufs=14))
            ppool = ctx.enter_context(tc.tile_pool(name="ppool", bufs=3))
            rpool = ctx.enter_context(tc.tile_pool(name="rpool", bufs=2))

            def big_slab(wd, m, name, eng=None):
                """[P, KX, P] slab of a [DM, *] weight, DMA split in two.

                eng picks the HWDGE queue: phase-B slabs ride the scalar
                engine's queue so they don't serialize behind the 8MB xT
                stream on the sync queue."""
                e = eng if eng is not None else nc.sync
                t = wpool.tile([P, KX, P], BF, tag="wqrs", name=name)
                src = wd[:, m * P:(m + 1) * P].rearrange("(ko p) m -> p ko m", p=P)
                e.dma_start(out=t[:, 0:16, :], in_=src[:, 0:16, :])
                e.dma_start(out=t[:, 16:KX, :], in_=src[:, 16:KX, :])
                return t

            def wo_slab(n):
                t = wpool.tile([P, 8, 512], BF, tag="wqrs", name=f"wos{n}")
                src = wo[:, n * 512:(n + 1) * 512].rearrange(
                    "(kc p) m -> p kc m", p=P
                )
                nc.sync.dma_start(out=t[:, 0:4, :], in_=src[:, 0:4, :])
                nc.sync.dma_start(out=t[:, 4:8, :], in_=src[:, 4:8, :])
                return t

            with ExitStack() as pctx:
                cpp = pctx.enter_context(
                    tc.tile_pool(name="cpp", bufs=2, space="PSUM")
                )
                spp = pctx.enter_context(
                    tc.tile_pool(name="spp", bufs=4, space="PSUM")
                )
                avp = pctx.enter_context(
                    tc.tile_pool(name="avp", bufs=2, space="PSUM")
                )

                # ---- Phase B: latent down-projections (feature-major) ----
                # First weight slab is DMA'd BEFORE the bulk xT stream so the
                # PE can start ~4us in instead of waiting out ~26us of DMA.
                items = [(wqd, cq_sb, m) for m in range(LC)] + [
                    (wkvd, ckv_sb, m) for m in range(LC)
                ]
                pend = big_slab(items[0][0], items[0][2], "bs0")
                for kq in range(4):
                    nc.sync.dma_start(
                        out=xT_sb[:, kq * 8:(kq + 1) * 8, :],
                        in_=xT_r[:, kq * 8:(kq + 1) * 8, :],
                    )
                nc.sync.dma_start(
                    out=wkr_sb[:], in_=wkr[:].rearrange("(ko p) d -> p ko d", p=P)
                )
                nc.sync.dma_start(
                    out=wvu_sb[:], in_=wvu[:].rearrange("(c p) m -> p c m", p=P)
                )
                for i, (wd, cdst, m) in enumerate(items):
                    slab = pend
                    if i + 1 < len(items):
                        nwd, _, nm = items[i + 1]
                        pend = big_slab(nwd, nm, f"bs{i + 1}")
                    # both hf chains advance per xT chunk: each chunk is
                    # consumed once, halving the demand rate on the still-
                    # streaming bulk xT DMA
                    pss2 = [
                        cpp.tile([P, 512], F32, tag="cps", name=f"b_ps{hf}")
                        for hf in range(2)
                    ]
                    for k in range(KX):
                        for hf in range(2):
                            nc.tensor.matmul(
                                pss2[hf][:],
                                slab[:, k, :],
                                xT_sb[:, k, hf * 512:(hf + 1) * 512],
                                start=(k == 0),
                                stop=(k == KX - 1),
                            )
                    for hf in range(2):
                        nc.vector.tensor_copy(
                            out=cdst[:, m, hf * 512:(hf + 1) * 512],
                            in_=pss2[hf][:],
                        )

                # k_rope^T [64, T]: the two token halves run as concurrent
                # col-tiles (M=64 at array cols 0 and 64) in one PSUM bank
                ps = cpp.tile([P, 512], F32, tag="cps", name="kr_ps")
                for k in range(KX):
                    for hf in range(2):
                        nc.tensor.matmul(
                            ps[hf * 64:hf * 64 + DR, :],
                            wkr_sb[:, k, :],
                            xT_sb[:, k, hf * 512:(hf + 1) * 512],
                            start=(k == 0),
                            stop=(k == KX - 1),
                            tile_position=(0, hf * 64),
                        )
                for hf in range(2):
                    nc.vector.tensor_copy(
                        out=kr_sb[:, hf * 512:(hf + 1) * 512],
                        in_=ps[hf * 64:hf * 64 + DR, :],
                    )

                # ---- Phase V: v = c_kv @ Wv_up (token-major) ----
                for tt in range(8):
                    for hf in range(2):
                        ps = cpp.tile([P, 512], F32, tag="cps", name="v_ps")
                        for lc in range(LC):
                            nc.tensor.matmul(
                                ps[:],
                                ckv_sb[:, lc, tt * P:(tt + 1) * P],
                                wvu_sb[:, lc, hf * 512:(hf + 1) * 512],
                                start=(lc == 0),
                                stop=(lc == LC - 1),
                            )
                        nc.vector.tensor_copy(
                            out=v_sb[:, tt, hf * 16:(hf + 1) * 16, 0:32],
                            in_=ps[:].rearrange("p (h d) -> p h d", h=16),
                        )

                # ---- projections for one head group, as a generator that
                # yields every ~2 matmuls (72 yields) so attention of group
                # g-1 can interleave them between its score matmuls ----
                def proj_gen(g, qt, kt):
                    slab = big_slab(wqr, 2 * g, f"qrs{g}_0")
                    nxt = None
                    for s in range(2):
                        qa = qkpool.tile([P, T], BF, tag="qkt", name=f"qt{g}_{2 * s}")
                        qb = qkpool.tile(
                            [P, T], BF, tag="qkt", name=f"qt{g}_{2 * s + 1}"
                        )
                        qt.append(qa)
                        qt.append(qb)
                        for hf in range(2):
                            ps = cpp.tile([P, 512], F32, tag="cps", name="qr_ps")
                            for k0 in range(0, KX, 2):
                                for k in (k0, k0 + 1):
                                    nc.tensor.matmul(
                                        ps[:],
                                        slab[:, k, :],
                                        xT_sb[:, k, hf * 512:(hf + 1) * 512],
                                        start=(k == 0),
                                        stop=(k == KX - 1),
                                    )
                                if s == 0 and hf == 1 and k0 == 16:
                                    nxt = big_slab(wqr, 2 * g + 1, f"qrs{g}_1")
                                yield
                            sl = slice(hf * 512, (hf + 1) * 512)
                            nc.vector.tensor_copy(out=qa[0:DR, sl], in_=ps[0:DR, :])
                            nc.vector.tensor_copy(out=qb[0:DR, sl], in_=ps[DR:P, :])
                        slab = nxt

                    for idx, (wu, dst, csrc) in enumerate(
                        ((wqu, qt, cq_sb), (wku, kt, ckv_sb))
                    ):
                        if idx == 1:
                            for j in range(4):
                                ktj = qkpool.tile(
                                    [P, T], BF, tag="qkt", name=f"kt{g}_{j}"
                                )
                                kt.append(ktj)
                                # SBUF->SBUF broadcast of the shared k_rope on
                                # the DMA queue: keeps the DVE free at group
                                # seams where the first scores of the next
                                # group wait on these tiles
                                nc.sync.dma_start(out=ktj[0:DR, :], in_=kr_sb[:])
                        ups = wpool.tile([P, LC, P], BF, tag="wups", name=f"up{g}_{idx}")
                        nc.sync.dma_start(
                            out=ups[:],
                            in_=wu[:, g * P:(g + 1) * P].rearrange(
                                "(c p) m -> p c m", p=P
                            ),
                        )
                        pss = []
                        for hf in range(2):
                            ps = cpp.tile([P, 512], F32, tag="cps", name="up_ps")
                            pss.append(ps)
                            for lc in range(LC):
                                nc.tensor.matmul(
                                    ps[:],
                                    ups[:, lc, :],
                                    csrc[:, lc, hf * 512:(hf + 1) * 512],
                                    start=(lc == 0),
                                    stop=(lc == LC - 1),
                                )
                                if lc == 1:
                                    yield
                            yield
                        # copies ordered head-first so head 0's operands are
                        # complete earliest (they gate the next group's first
                        # score matmuls)
                        for j in range(4):
                            for hf in range(2):
                                nc.vector.tensor_copy(
                                    out=dst[j][DR:DR + DH,
                                               hf * 512:(hf + 1) * 512],
                                    in_=pss[hf][j * DH:(j + 1) * DH, :],
                                )

                # ---- attention, with next group's projections interleaved ----
                qts = {0: []}
                kts = {0: []}
                g0 = proj_gen(0, qts[0], kts[0])
                for _ in g0:
                    pass
                gen = None
                woslabs = []
                wo_ps = []

                def wo_fill_gen():
                    # Group 7 has no next-group projections, so without
                    # filler the PE idles ~1-2us per head and the HAM clock
                    # gate drops to 4/8 for the rest of the kernel tail
                    # (measured: 51us at half clock).  Open the first Wo
                    # slab's first two token-block chains (head groups 0..6
                    # are final) in the otherwise-idle cpp banks to keep the
                    # PE warm; kc=7 joins after the last group completes.
                    for tt in range(2):
                        ps = cpp.tile([P, 512], F32, tag="cps", name=f"wof{tt}")
                        wo_ps.append(ps)
                        for kc in range(7):
                            nc.tensor.matmul(
                                ps[:],
                                aout_sb[:, kc, tt * P:(tt + 1) * P],
                                woslabs[0][:, kc, :],
                                start=(kc == 0),
                                stop=(kc == 6),
                            )
                            yield

                for g in range(8):
                    if g + 1 < 8:
                        qts[g + 1] = []
                        kts[g + 1] = []
                        gen = proj_gen(g + 1, qts[g + 1], kts[g + 1])
                    else:
                        # prefetch first two Wo slabs during the last group
                        woslabs = [wo_slab(0), wo_slab(1)]
                        gen = wo_fill_gen()

                    def pull():
                        if gen is not None:
                            next(gen, None)

                    qt, kt = qts[g], kts[g]
                    for j in range(4):
                        h = 4 * g + j
                        probs = [
                            ppool.tile(
                                [P, 8, 512], BF, tag="probs", name=f"pb{g}_{j}_{qh}"
                            )
                            for qh in range(2)
                        ]
                        for qh in range(2):
                            for kc in range(8):
                                sp = spp.tile([P, 512], F32, tag="sps", name="sps")
                                nc.tensor.matmul(
                                    sp[:],
                                    kt[j][0:96, kc * P:(kc + 1) * P],
                                    qt[j][0:96, qh * 512:(qh + 1) * 512],
                                    start=True,
                                    stop=True,
                                )
                                pull()
                                nc.scalar.activation(
                                    out=probs[qh][:, kc, :],
                                    in_=sp[:],
                                    func=mybir.ActivationFunctionType.Exp,
                                    scale=SCALE,
                                )
                        # attn@V for both query halves concurrently: the two
                        # M=33 accumulation chains col-tile the PE array
                        # (cols 0-32 and 64-96), sharing one PSUM bank.
                        av = avp.tile([P, 512], F32, tag="avp", name="av")
                        for kc in range(8):
                            for qh in range(2):
                                nc.tensor.matmul(
                                    av[qh * 64:qh * 64 + 33, :],
                                    v_sb[:, kc, h, 0:33],
                                    probs[qh][:, kc, :],
                                    start=(kc == 0),
                                    stop=(kc == 7),
                                    tile_position=(0, qh * 64),
                                )
                            if kc % 4 == 3:
                                pull()
                        for qh in range(2):
                            # ACT stages the denominator to SBUF partition 0:
                            # reciprocal_approx_fast requires base partition 0,
                            # and gpsimd (broadcast) cannot read PSUM.
                            den = rpool.tile([1, 512], F32, tag="rc", name="den")
                            nc.scalar.copy(out=den[:], in_=av[qh * 64 + 32:qh * 64 + 33, :])
                            recip = rpool.tile([1, 512], F32, tag="rc", name="recip")
                            nc.vector.reciprocal_approx_fast(recip[:], den[:])
                            rrep = rpool.tile([DH, 512], F32, tag="rr", name="rrep")
                            nc.gpsimd.partition_broadcast(rrep[:], recip[:])
                            nc.vector.tensor_mul(
                                out=aout_sb[
                                    j * DH:(j + 1) * DH, g, qh * 512:(qh + 1) * 512
                                ],
                                in0=av[qh * 64:qh * 64 + DH, :],
                                in1=rrep[:],
                            )
                            pull()
                    if gen is not None:
                        for _ in gen:
                            pass

                # ---- Wo slab 0 finishes inside the attention PSUM scope:
                # kc=7 joins the two warm-up chains, and token blocks 2-7 run
                # on the attention banks as they free up - the PE crosses
                # into phase E without an idle window (keeps HAM at 8/8).
                woslabs.append(wo_slab(2))
                for tt in range(8):
                    if tt < 2:
                        ps = wo_ps[tt]
                    else:
                        pool, tag = (spp, "sps") if tt < 6 else (avp, "avp")
                        ps = pool.tile([P, 512], F32, tag=tag, name=f"e0_{tt}")
                        for kc in range(7):
                            nc.tensor.matmul(
                                ps[:],
                                aout_sb[:, kc, tt * P:(tt + 1) * P],
                                woslabs[0][:, kc, :],
                                start=(kc == 0),
                                stop=(kc == 6),
                            )
                    nc.tensor.matmul(
                        ps[:],
                        aout_sb[:, 7, tt * P:(tt + 1) * P],
                        woslabs[0][:, 7, :],
                        start=False,
                        stop=True,
                        skip_group_check=True,
                    )
                    ot0 = ppool.tile([P, 512], BF, tag="probs", name=f"eo0_{tt}")
                    nc.any.tensor_copy(out=ot0[:], in_=ps[:])
                    nc.scalar.dma_start(
                        out=out[tt * P:(tt + 1) * P, 0:512], in_=ot0[:]
                    )

            # ---- Phase E: out = aout^T @ Wo (token-major), PSUM->HBM direct
            with ExitStack() as ectx:
                epp = ectx.enter_context(
                    tc.tile_pool(name="epp", bufs=8, space="PSUM")
                )
                slabs = list(woslabs)
                for n in range(1, 8):
                    t = slabs[n]
                    pss = [
                        epp.tile([P, 512], F32, tag="eps", name=f"eps_{n}_{i}")
                        for i in range(8)
                    ]
                    for tt in range(8):
                        for kc in range(8):
                            nc.tensor.matmul(
                                pss[tt][:],
                                aout_sb[:, kc, tt * P:(tt + 1) * P],
                                t[:, kc, :],
                                start=(kc == 0),
                                stop=(kc == 7),
                            )
                    if n + 2 < 8:
                        slabs.append(wo_slab(n + 2))
                    for tt in range(8):
                        ot = ppool.tile([P, 512], BF, tag="probs", name=f"eo{n}_{tt}")
                        nc.any.tensor_copy(out=ot[:], in_=pss[tt][:])
                        # scalar HWDGE queue: ACT is idle during phase E and
                        # this keeps the sync queue free for Wo slab loads
                        nc.scalar.dma_start(
                            out=out[tt * P:(tt + 1) * P, n * 512:(n + 1) * 512],
                            in_=ot[:],
                        )

    nc.compile()
    return nc


def _prep_inputs(inputs):
    bf = ml_dtypes.bfloat16
    x = np.asarray(inputs["x"], dtype=np.float32)
    Wq_down = np.asarray(inputs["Wq_down"], dtype=np.float32).astype(bf)
    Wkv_down = np.asarray(inputs["Wkv_down"], dtype=np.float32).astype(bf)
    Wq_up = np.asarray(inputs["Wq_up"], dtype=np.float32).astype(bf)
    Wk_up = np.asarray(inputs["Wk_up"], dtype=np.float32).astype(bf)
    Wv_up = np.asarray(inputs["Wv_up"], dtype=np.float32).astype(bf)
    Wq_rope = np.asarray(inputs["Wq_rope"], dtype=np.float32).astype(bf)
    Wk_rope = np.asarray(inputs["Wk_rope"], dtype=np.float32).astype(bf)
    Wo = np.asarray(inputs["Wo"], dtype=np.float32).astype(bf)

    xT = [np.ascontiguousarray(x[b].T).astype(bf) for b in range(NB)]

    in_maps = []
    for core in range(8):
        b = core // 4
        hg = core % 4
        hs = slice(hg * NHC * DH, (hg + 1) * NHC * DH)        # head-dim cols
        rs = slice(hg * NHC * DR, (hg + 1) * NHC * DR)        # rope cols
        in_maps.append(
            {
                "xT": xT[b],
                "wqd": Wq_down,
                "wkvd": Wkv_down,
                "wqu": np.ascontiguousarray(Wq_up[:, hs]),
                "wku": np.ascontiguousarray(Wk_up[:, hs]),
                "wvu": np.ascontiguousarray(Wv_up[:, hs]),
                "wqr": np.ascontiguousarray(Wq_rope[:, rs]),
                "wkr": Wk_rope,
                "wo": np.ascontiguousarray(Wo[hs, :]),
            }
        )
    return in_maps


def kernel(**inputs):
    if "nc" not in _CACHE:
        _CACHE["nc"] = _build_program()
    nc = _CACHE["nc"]
    in_maps = _prep_inputs(inputs)
    res = run_bass_kernel_spmd(nc, in_maps, list(range(8)))
    out = np.zeros((NB, T, DM), dtype=np.float32)
    for core in range(8):
        out[core // 4] += res.results[core]["out"].astype(np.float32)
    return out

